# revision 15
# baseline (speedup 1.0000x reference)
"""Trainium2 Bass kernel for nn_D2FAgg (block-diagonal GNN message passing).

Sharding: B*N = 24576 output rows -> 24 chunks of 1024 rows; 3 chunks/core
across 8 cores. Each chunk belongs to one (batch, modality) block of 2048
nodes.

Host prep folds the masked L1 row-normalization into the edge block:
  eTs = (e_blk * diag_mask / rowsum_l1).T * S   quantized to fp8 e4m3
(1/S is folded into W_raw / u2). The device then computes, per chunk:
  aggTs[c, row] = sum_j xb8[j, c] * eTs[j, row]    (PE fp8 DoubleRow, K=2048)
  psum_d[row,c'] = xt.T@W_f + aggTs.T@(-W_r/S) + (b_f - b_r)   (PE bf16)
  psum_a[row,c'] = aggTs.T@(W_r/S) + b_r                        (PE bf16)
  beta[row] = sigmoid(m1 + m2 + K)   (PE matvecs + ACT)
  h = psum_a + beta * psum_d;  out = relu(LN(h))  (DVE STT + bn_stats, ACT)
"""
import numpy as np
import ml_dtypes
from contextlib import ExitStack

import concourse.bacc as bacc
import concourse.mybir as mybir
import concourse.tile as tile
from concourse.bass_utils import run_bass_kernel_spmd

F32 = mybir.dt.float32
F32R = mybir.dt.float32r
BF16 = mybir.dt.bfloat16
F8 = mybir.dt.float8e4
AF = mybir.ActivationFunctionType
ALU = mybir.AluOpType
DR = mybir.MatmulPerfMode.DoubleRow

NP_F8 = ml_dtypes.float8_e4m3
NP_BF16 = ml_dtypes.bfloat16

B, N, C = 4, 6144, 256
M = 3
n = N // M                      # 2048 nodes per modality block
NCORES = 8
RPC = 1024                      # rows per chunk
CPC = (B * N) // (NCORES * RPC)  # chunks per core = 3
NK = n // 128                   # 16 j-tiles per chunk
NT = RPC // 128                 # 8 row-tiles per chunk
NPC = 4                         # eT DMA pieces per chunk (4 k-tiles each)
EPS_L1, EPS_LN = 1e-12, 1e-5
S = 2048.0                      # fp8 pre-scale for normalized edges

_cache = {}


def _build(ln_trivial: bool):
    nc = bacc.Bacc("TRN2", target_bir_lowering=False, debug=False,
                   num_devices=NCORES)
    eTd = nc.declare_dram_parameter("eTd", [CPC, 128, NK, RPC], F8,
                                    isOutput=False)
    xbd = nc.declare_dram_parameter("xbd", [CPC, 128, NK, C], F8,
                                    isOutput=False)
    xtd = nc.declare_dram_parameter("xtd", [CPC, 128, 2, RPC], BF16,
                                    isOutput=False)
    wfd = nc.declare_dram_parameter("wfd", [128, CPC, 2, C], BF16,
                                    isOutput=False)
    wdd = nc.declare_dram_parameter("wdd", [128, CPC, 2, C], BF16,
                                    isOutput=False)
    wad = nc.declare_dram_parameter("wad", [128, CPC, 2, C], BF16,
                                    isOutput=False)
    u1d = nc.declare_dram_parameter("u1d", [128, CPC, 2, 2], BF16,
                                    isOutput=False)
    u2d = nc.declare_dram_parameter("u2d", [128, CPC, 2, 2], BF16,
                                    isOutput=False)
    kbd = nc.declare_dram_parameter("kbd", [128, CPC], F32, isOutput=False)
    bzd = nc.declare_dram_parameter("bzd", [1, CPC, 2, C], BF16,
                                    isOutput=False)
    onesr = nc.declare_dram_parameter("onesr", [1, 128], BF16, isOutput=False)
    if not ln_trivial:
        gmd = nc.declare_dram_parameter("gmd", [128, CPC, C], F32,
                                        isOutput=False)
        btd = nc.declare_dram_parameter("btd", [128, CPC, C], F32,
                                        isOutput=False)
    out = nc.declare_dram_parameter("out", [CPC, 128, NT, C], BF16,
                                    isOutput=True)

    with ExitStack() as ctx:
        tc = ctx.enter_context(tile.TileContext(nc))
        const = ctx.enter_context(tc.tile_pool(name="const", bufs=1))
        px = ctx.enter_context(tc.tile_pool(name="px", bufs=2))
        pe_pool = ctx.enter_context(tc.tile_pool(name="pe", bufs=8))
        pag = ctx.enter_context(tc.tile_pool(name="pag", bufs=2))
        pwork = ctx.enter_context(tc.tile_pool(name="pwork", bufs=4))
        pout = ctx.enter_context(tc.tile_pool(name="pout", bufs=2))
        ps_big = ctx.enter_context(tc.tile_pool(name="psbig", bufs=4,
                                                space="PSUM"))
        ps_sm = ctx.enter_context(tc.tile_pool(name="pssm", bufs=1,
                                               space="PSUM"))
        ps_da = ctx.enter_context(tc.tile_pool(name="psda", bufs=3,
                                               space="PSUM"))

        # once-loaded constants / weights
        ones_sb = const.tile([1, 128], BF16)
        nc.gpsimd.dma_start(ones_sb[:], onesr[:])
        eps_t = const.tile([128, 1], F32)
        nc.vector.memset(eps_t[:], EPS_LN)
        wf_sb = const.tile([128, CPC, 2, C], BF16)
        nc.gpsimd.dma_start(wf_sb[:], wfd[:])
        wd_sb = const.tile([128, CPC, 2, C], BF16)
        nc.gpsimd.dma_start(wd_sb[:], wdd[:])
        wa_sb = const.tile([128, CPC, 2, C], BF16)
        nc.gpsimd.dma_start(wa_sb[:], wad[:])
        u1_sb = const.tile([128, CPC, 2, 2], BF16)
        nc.gpsimd.dma_start(u1_sb[:], u1d[:])
        u2_sb = const.tile([128, CPC, 2, 2], BF16)
        nc.gpsimd.dma_start(u2_sb[:], u2d[:])
        kb_sb = const.tile([128, CPC], F32)
        nc.gpsimd.dma_start(kb_sb[:], kbd[:])
        bz_sb = const.tile([1, CPC, 2, C], BF16)
        nc.gpsimd.dma_start(bz_sb[:], bzd[:])
        if not ln_trivial:
            gm_sb = const.tile([128, CPC, C], F32)
            nc.gpsimd.dma_start(gm_sb[:], gmd[:])
            bt_sb = const.tile([128, CPC, C], F32)
            nc.gpsimd.dma_start(bt_sb[:], btd[:])

        for k in range(CPC):
            xb_sb = px.tile([128, NK, C], F8, tag="xb")
            nc.sync.dma_start(xb_sb[:], xbd[k])

            # ---- phase A: scaled-normalized aggrT via fp8 DoubleRow ----
            agg_ps = [[ps_big.tile([128, 512], F32, tag="agg",
                                   name=f"agg_{k}_{h}_{rh}")
                       for rh in range(2)] for h in range(2)]
            ets = []
            for pc in range(NPC):
                et = pe_pool.tile([128, 4, RPC], F8, tag="et")
                nc.sync.dma_start(et[:], eTd[k][:, 4 * pc:4 * pc + 4, :])
                ets.append(et)
            # xt after the edge pieces: only needed in phase B
            xt_sb = px.tile([128, 2, RPC], BF16, tag="xt")
            nc.sync.dma_start(xt_sb[:], xtd[k])
            for pc in range(NPC):
                et = ets[pc]
                for jj in range(2):
                    kt = 4 * pc + 2 * jj
                    for h in range(2):
                        for rh in range(2):
                            nc.tensor.matmul(
                                agg_ps[h][rh][:],
                                xb_sb[:, kt:kt + 2, h * 128:(h + 1) * 128],
                                et[:, 2 * jj:2 * jj + 2,
                                   rh * 512:(rh + 1) * 512],
                                start=(pc == 0 and jj == 0),
                                stop=(pc == NPC - 1 and jj == 1),
                                perf_mode=DR)

            # aggrT (scaled) -> SBUF bf16 (split across ACT and DVE)
            agT = pag.tile([128, 2, RPC], BF16, tag="agT")
            for h in range(2):
                nc.scalar.copy(agT[:, h, 0:512], agg_ps[h][0][:])
                nc.vector.tensor_copy(agT[:, h, 512:1024], agg_ps[h][1][:])

            # ---- gate: beta = sigmoid(m1 + m2 + K); m1+m2 summed in PSUM ----
            m_ps = ps_sm.tile([128, 2 * NT], F32, tag="sm")
            for t in range(NT):
                sl = slice(t * 128, (t + 1) * 128)
                nc.tensor.matmul(m_ps[:, 2 * t:2 * t + 2],
                                 xt_sb[:, 0, sl], u1_sb[:, k, 0, :],
                                 start=True, stop=False)
                nc.tensor.matmul(m_ps[:, 2 * t:2 * t + 2],
                                 xt_sb[:, 1, sl], u1_sb[:, k, 1, :],
                                 start=False, stop=False)
                nc.tensor.matmul(m_ps[:, 2 * t:2 * t + 2],
                                 agT[:, 0, sl], u2_sb[:, k, 0, :],
                                 start=False, stop=False)
                nc.tensor.matmul(m_ps[:, 2 * t:2 * t + 2],
                                 agT[:, 1, sl], u2_sb[:, k, 1, :],
                                 start=False, stop=True)
            beta_sb = pwork.tile([128, NT], F32, tag="beta")
            nc.scalar.activation(beta_sb[:], m_ps[:, 0:2 * NT:2], AF.Sigmoid,
                                 bias=kb_sb[:, k:k + 1])

            # ---- per row-tile: fused projections + gate + LN stats ----
            mv = pwork.tile([128, 2 * NT], F32, tag="mv")
            h_all = pout.tile([128, NT, C], F32, tag="hall")
            for t in range(NT):
                sl = slice(t * 128, (t + 1) * 128)
                da = ps_da.tile([128, 2, C], F32, tag="da")
                pd = da[:, 0, :]
                pa = da[:, 1, :]
                nc.tensor.matmul(pd[:], xt_sb[:, 0, sl], wf_sb[:, k, 0, :],
                                 start=True, stop=False)
                nc.tensor.matmul(pd[:], xt_sb[:, 1, sl], wf_sb[:, k, 1, :],
                                 start=False, stop=False)
                nc.tensor.matmul(pd[:], agT[:, 0, sl], wd_sb[:, k, 0, :],
                                 start=False, stop=False)
                nc.tensor.matmul(pd[:], agT[:, 1, sl], wd_sb[:, k, 1, :],
                                 start=False, stop=False)
                nc.tensor.matmul(pd[:], ones_sb[:], bz_sb[:, k, 0, :],
                                 start=False, stop=True)
                nc.tensor.matmul(pa[:], agT[:, 0, sl], wa_sb[:, k, 0, :],
                                 start=True, stop=False)
                nc.tensor.matmul(pa[:], agT[:, 1, sl], wa_sb[:, k, 1, :],
                                 start=False, stop=False)
                nc.tensor.matmul(pa[:], ones_sb[:], bz_sb[:, k, 1, :],
                                 start=False, stop=True)
                pa_sb = pwork.tile([128, C], F32, tag="pasb")
                nc.scalar.copy(pa_sb[:], pa[:])
                nc.vector.scalar_tensor_tensor(h_all[:, t, :], pd[:],
                                               beta_sb[:, t:t + 1], pa_sb[:],
                                               ALU.mult, ALU.add)
                stats = pwork.tile([128, 6], F32, tag="stats")
                nc.vector.bn_stats(stats[:], h_all[:, t, :])
                nc.vector.bn_aggr(mv[:, 2 * t:2 * t + 2], stats[:])

            # LN tail in two halves so the first half's relus + output DMA
            # drain while the second half's bn stats are still in flight.
            out_sb = pout.tile([128, NT, C], BF16, tag="out")
            HH = NT // 2
            for hlf in range(2):
                t0 = hlf * HH
                sd = pwork.tile([128, HH], F32, tag=f"sd{hlf}")
                nc.scalar.activation(sd[:], mv[:, 2 * t0 + 1:2 * (t0 + HH):2],
                                     AF.Sqrt, bias=eps_t[:, 0:1])
                rs2 = pwork.tile([128, HH], F32, tag=f"rs2{hlf}")
                nc.vector.reciprocal(rs2[:], sd[:])
                ms = pwork.tile([128, HH], F32, tag=f"ms{hlf}")
                nc.vector.scalar_tensor_tensor(
                    ms[:], mv[:, 2 * t0:2 * (t0 + HH):2], -1.0, rs2[:],
                    ALU.mult, ALU.mult)
                for i in range(HH):
                    t = t0 + i
                    if ln_trivial:
                        nc.scalar.activation(out_sb[:, t, :], h_all[:, t, :],
                                             AF.Relu, bias=ms[:, i:i + 1],
                                             scale=rs2[:, i:i + 1])
                    else:
                        z_t = pwork.tile([128, C], F32, tag="z")
                        nc.scalar.activation(z_t[:], h_all[:, t, :], AF.Copy,
                                             bias=0.0, scale=rs2[:, i:i + 1])
                        zb = pwork.tile([128, C], F32, tag="zb")
                        nc.vector.tensor_scalar(zb[:], z_t[:], ms[:, i:i + 1],
                                                None, ALU.add)
                        zg = pwork.tile([128, C], F32, tag="zg")
                        nc.vector.tensor_tensor(zg[:], zb[:], gm_sb[:, k, :],
                                                ALU.mult)
                        za = pwork.tile([128, C], F32, tag="za")
                        nc.vector.tensor_tensor(za[:], zg[:], bt_sb[:, k, :],
                                                ALU.add)
                        nc.vector.tensor_scalar_max(out_sb[:, t, :], za[:],
                                                    0.0)
                # out DMA on the ACT queue: emitted right after this half's
                # relus (same engine) so it never stalls the SP input queue
                nc.scalar.dma_start(out[k][:, t0:t0 + HH, :],
                                    out_sb[:, t0:t0 + HH, :])

    nc.compile()
    return nc


def _prep_inputs(distribution_edge, feature_node, modal_id, W_feat, b_feat,
                 W_raw, b_raw, W_beta, b_beta, ln_gamma, ln_beta):
    de = np.ascontiguousarray(distribution_edge, dtype=np.float32)
    x = np.ascontiguousarray(feature_node, dtype=np.float32)
    Wf = np.asarray(W_feat, np.float32)
    bf = np.asarray(b_feat, np.float32)
    Wr = np.asarray(W_raw, np.float32)
    br = np.asarray(b_raw, np.float32)
    Wb = np.asarray(W_beta, np.float32)
    bb = np.asarray(b_beta, np.float32)
    g = np.asarray(ln_gamma, np.float32)
    be = np.asarray(ln_beta, np.float32)

    ln_trivial = bool(np.all(g == 1.0) and np.all(be == 0.0))

    # folded gate params (1/S folded into u2)
    u1 = np.stack([Wf[i] @ (Wb[i][:C] + Wb[i][2 * C:]) for i in range(M)])
    u2 = np.stack([Wr[i] @ (Wb[i][C:2 * C] - Wb[i][2 * C:]) / S
                   for i in range(M)])
    kk = np.array([bb[i] + bf[i] @ (Wb[i][:C] + Wb[i][2 * C:])
                   + br[i] @ (Wb[i][C:2 * C] - Wb[i][2 * C:])
                   for i in range(M)], np.float32)

    halves = n // RPC  # 2 chunks per block
    rr = np.arange(RPC)
    in_maps = []
    for c in range(NCORES):
        eT_c = np.empty((CPC, 128, NK, RPC), NP_F8)
        xb_c = np.empty((CPC, 128, NK, C), NP_F8)
        xt_c = np.empty((CPC, 128, 2, RPC), NP_BF16)
        wf_c = np.empty((128, CPC, 2, C), NP_BF16)
        wd_c = np.empty((128, CPC, 2, C), NP_BF16)
        wa_c = np.empty((128, CPC, 2, C), NP_BF16)
        u1_c = np.zeros((128, CPC, 2, 2), NP_BF16)
        u2_c = np.zeros((128, CPC, 2, 2), NP_BF16)
        kb_c = np.empty((128, CPC), np.float32)
        bz_c = np.empty((1, CPC, 2, C), NP_BF16)
        gm_c = np.empty((128, CPC, C), np.float32)
        bt_c = np.empty((128, CPC, C), np.float32)
        for k in range(CPC):
            g_idx = c * CPC + k               # global chunk id
            b_idx = g_idx // (M * halves)
            i_idx = (g_idx // halves) % M
            half = g_idx % halves
            r0 = i_idx * n + half * RPC       # first global row in batch b
            blk = de[b_idx, r0:r0 + RPC,
                     i_idx * n:(i_idx + 1) * n].copy()  # [RPC, n]
            blk[rr, half * RPC + rr] = 0.0    # zero self-edges
            rs = np.maximum(np.abs(blk).sum(axis=1), EPS_L1)
            eTs = (blk * (S / rs)[:, None]).T           # [n(j), RPC(rows)]
            eT_c[k] = eTs.astype(NP_F8).reshape(NK, 128, RPC).transpose(1, 0, 2)
            xblk = x[b_idx, i_idx * n:(i_idx + 1) * n, :]
            xb_c[k] = xblk.astype(NP_F8).reshape(NK, 128, C).transpose(1, 0, 2)
            xt_c[k] = (x[b_idx, r0:r0 + RPC, :].T.astype(NP_BF16)
                       .reshape(2, 128, RPC).transpose(1, 0, 2))
            wf_c[:, k] = Wf[i_idx].astype(NP_BF16).reshape(2, 128, C).transpose(1, 0, 2)
            wd_c[:, k] = (-Wr[i_idx] / S).astype(NP_BF16).reshape(2, 128, C).transpose(1, 0, 2)
            wa_c[:, k] = (Wr[i_idx] / S).astype(NP_BF16).reshape(2, 128, C).transpose(1, 0, 2)
            u1_c[:, k, :, 0] = u1[i_idx].astype(NP_BF16).reshape(2, 128).T
            u2_c[:, k, :, 0] = u2[i_idx].astype(NP_BF16).reshape(2, 128).T
            kb_c[:, k] = kk[i_idx]
            bz_c[0, k, 0] = (bf[i_idx] - br[i_idx]).astype(NP_BF16)
            bz_c[0, k, 1] = br[i_idx].astype(NP_BF16)
            gm_c[:, k] = g[i_idx][None, :]
            bt_c[:, k] = be[i_idx][None, :]
        im = dict(eTd=eT_c, xbd=xb_c, xtd=xt_c, wfd=wf_c, wdd=wd_c,
                  wad=wa_c, u1d=u1_c, u2d=u2_c, kbd=kb_c, bzd=bz_c,
                  onesr=np.ones((1, 128), NP_BF16))
        if not ln_trivial:
            im["gmd"] = gm_c
            im["btd"] = bt_c
        in_maps.append(im)
    return in_maps, ln_trivial


def kernel(**inputs) -> np.ndarray:
    in_maps, ln_trivial = _prep_inputs(**inputs)
    if ln_trivial not in _cache:
        _cache[ln_trivial] = _build(ln_trivial)
    nc = _cache[ln_trivial]
    res = run_bass_kernel_spmd(nc, in_maps, core_ids=list(range(NCORES)))
    out = np.empty((B * N, C), np.float32)
    for c in range(NCORES):
        o = np.asarray(res.results[c]["out"])  # [CPC, 128, NT, C] bf16
        o = o.astype(np.float32).transpose(0, 2, 1, 3).reshape(CPC * RPC, C)
        out[c * CPC * RPC:(c + 1) * CPC * RPC] = o
    return out.reshape(B, N, C)


# revision 16
# speedup vs baseline: 1.1002x; 1.1002x over previous
"""Trainium2 Bass kernel for nn_D2FAgg (block-diagonal GNN message passing).

Sharding: B*N = 24576 output rows -> 24 chunks of 1024 rows; 3 chunks/core
across 8 cores. Each chunk belongs to one (batch, modality) block of 2048
nodes.

Host prep folds the masked L1 row-normalization into the edge block:
  eTs = (e_blk * diag_mask / rowsum_l1).T * S   quantized to fp8 e4m3
(1/S is folded into W_raw / u2). The device then computes, per chunk:
  aggTs[c, row] = sum_j xb8[j, c] * eTs[j, row]    (PE fp8 DoubleRow, K=2048)
  psum_d[row,c'] = xt.T@W_f + aggTs.T@(-W_r/S) + (b_f - b_r)   (PE bf16)
  psum_a[row,c'] = aggTs.T@(W_r/S) + b_r                        (PE bf16)
  beta[row] = sigmoid(m1 + m2 + K)   (PE matvecs + ACT)
  h = psum_a + beta * psum_d;  out = relu(LN(h))  (DVE STT + bn_stats, ACT)
"""
import numpy as np
import ml_dtypes
from contextlib import ExitStack

import concourse.bacc as bacc
import concourse.mybir as mybir
import concourse.tile as tile
from concourse.bass_utils import run_bass_kernel_spmd

F32 = mybir.dt.float32
F32R = mybir.dt.float32r
BF16 = mybir.dt.bfloat16
F8 = mybir.dt.float8e4
AF = mybir.ActivationFunctionType
ALU = mybir.AluOpType
DR = mybir.MatmulPerfMode.DoubleRow

NP_F8 = ml_dtypes.float8_e4m3
NP_BF16 = ml_dtypes.bfloat16

B, N, C = 4, 6144, 256
M = 3
n = N // M                      # 2048 nodes per modality block
NCORES = 8
RPC = 1024                      # rows per chunk
CPC = (B * N) // (NCORES * RPC)  # chunks per core = 3
NK = n // 128                   # 16 j-tiles per chunk
NT = RPC // 128                 # 8 row-tiles per chunk
NPC = 4                         # eT DMA pieces per chunk (4 k-tiles each)
EPS_L1, EPS_LN = 1e-12, 1e-5
S = 2048.0                      # fp8 pre-scale for normalized edges

_cache = {}


def _build(ln_trivial: bool):
    nc = bacc.Bacc("TRN2", target_bir_lowering=False, debug=False,
                   num_devices=NCORES)
    eTd = nc.declare_dram_parameter("eTd", [CPC, 128, NK, RPC], F8,
                                    isOutput=False)
    xbd = nc.declare_dram_parameter("xbd", [CPC, 128, NK, C], F8,
                                    isOutput=False)
    xtd = nc.declare_dram_parameter("xtd", [CPC, 128, 2, RPC], BF16,
                                    isOutput=False)
    wfd = nc.declare_dram_parameter("wfd", [128, CPC, 2, C], BF16,
                                    isOutput=False)
    wdd = nc.declare_dram_parameter("wdd", [128, CPC, 2, C], BF16,
                                    isOutput=False)
    wad = nc.declare_dram_parameter("wad", [128, CPC, 2, C], BF16,
                                    isOutput=False)
    u1d = nc.declare_dram_parameter("u1d", [128, CPC, 2, 2], BF16,
                                    isOutput=False)
    u2d = nc.declare_dram_parameter("u2d", [128, CPC, 2, 2], BF16,
                                    isOutput=False)
    kbd = nc.declare_dram_parameter("kbd", [128, CPC], F32, isOutput=False)
    bzd = nc.declare_dram_parameter("bzd", [1, CPC, 2, C], BF16,
                                    isOutput=False)
    onesr = nc.declare_dram_parameter("onesr", [1, 128], BF16, isOutput=False)
    if not ln_trivial:
        gmd = nc.declare_dram_parameter("gmd", [128, CPC, C], F32,
                                        isOutput=False)
        btd = nc.declare_dram_parameter("btd", [128, CPC, C], F32,
                                        isOutput=False)
    out = nc.declare_dram_parameter("out", [CPC, 128, NT, C], BF16,
                                    isOutput=True)

    with ExitStack() as ctx:
        tc = ctx.enter_context(tile.TileContext(nc))
        const = ctx.enter_context(tc.tile_pool(name="const", bufs=1))
        px = ctx.enter_context(tc.tile_pool(name="px", bufs=2))
        pe_pool = ctx.enter_context(tc.tile_pool(name="pe", bufs=8))
        pag = ctx.enter_context(tc.tile_pool(name="pag", bufs=2))
        pwork = ctx.enter_context(tc.tile_pool(name="pwork", bufs=4))
        pout = ctx.enter_context(tc.tile_pool(name="pout", bufs=2))
        ps_big = ctx.enter_context(tc.tile_pool(name="psbig", bufs=4,
                                                space="PSUM"))
        ps_sm = ctx.enter_context(tc.tile_pool(name="pssm", bufs=1,
                                               space="PSUM"))
        ps_da = ctx.enter_context(tc.tile_pool(name="psda", bufs=3,
                                               space="PSUM"))

        # once-loaded constants / weights
        ones_sb = const.tile([1, 128], BF16)
        nc.scalar.dma_start(ones_sb[:], onesr[:])
        eps_t = const.tile([128, 1], F32)
        nc.vector.memset(eps_t[:], EPS_LN)
        wf_sb = const.tile([128, CPC, 2, C], BF16)
        nc.scalar.dma_start(wf_sb[:], wfd[:])
        wd_sb = const.tile([128, CPC, 2, C], BF16)
        nc.scalar.dma_start(wd_sb[:], wdd[:])
        wa_sb = const.tile([128, CPC, 2, C], BF16)
        nc.scalar.dma_start(wa_sb[:], wad[:])
        u1_sb = const.tile([128, CPC, 2, 2], BF16)
        nc.scalar.dma_start(u1_sb[:], u1d[:])
        u2_sb = const.tile([128, CPC, 2, 2], BF16)
        nc.scalar.dma_start(u2_sb[:], u2d[:])
        kb_sb = const.tile([128, CPC], F32)
        nc.scalar.dma_start(kb_sb[:], kbd[:])
        bz_sb = const.tile([1, CPC, 2, C], BF16)
        nc.scalar.dma_start(bz_sb[:], bzd[:])
        if not ln_trivial:
            gm_sb = const.tile([128, CPC, C], F32)
            nc.scalar.dma_start(gm_sb[:], gmd[:])
            bt_sb = const.tile([128, CPC, C], F32)
            nc.scalar.dma_start(bt_sb[:], btd[:])

        for k in range(CPC):
            xb_sb = px.tile([128, NK, C], F8, tag="xb")
            nc.sync.dma_start(xb_sb[:], xbd[k])

            # ---- phase A: scaled-normalized aggrT via fp8 DoubleRow ----
            agg_ps = [[ps_big.tile([128, 512], F32, tag="agg",
                                   name=f"agg_{k}_{h}_{rh}")
                       for rh in range(2)] for h in range(2)]
            ets = []
            for pc in range(NPC):
                et = pe_pool.tile([128, 4, RPC], F8, tag="et")
                nc.sync.dma_start(et[:], eTd[k][:, 4 * pc:4 * pc + 4, :])
                ets.append(et)
            # xt after the edge pieces: only needed in phase B
            xt_sb = px.tile([128, 2, RPC], BF16, tag="xt")
            nc.sync.dma_start(xt_sb[:], xtd[k])
            for pc in range(NPC):
                et = ets[pc]
                for jj in range(2):
                    kt = 4 * pc + 2 * jj
                    for h in range(2):
                        for rh in range(2):
                            nc.tensor.matmul(
                                agg_ps[h][rh][:],
                                xb_sb[:, kt:kt + 2, h * 128:(h + 1) * 128],
                                et[:, 2 * jj:2 * jj + 2,
                                   rh * 512:(rh + 1) * 512],
                                start=(pc == 0 and jj == 0),
                                stop=(pc == NPC - 1 and jj == 1),
                                perf_mode=DR)

            # aggrT (scaled) -> SBUF bf16 (split across ACT and DVE)
            agT = pag.tile([128, 2, RPC], BF16, tag="agT")
            for h in range(2):
                nc.scalar.copy(agT[:, h, 0:512], agg_ps[h][0][:])
                nc.vector.tensor_copy(agT[:, h, 512:1024], agg_ps[h][1][:])

            # ---- gate: beta = sigmoid(m1 + m2 + K); m1+m2 summed in PSUM ----
            m_ps = ps_sm.tile([128, 2 * NT], F32, tag="sm")
            for t in range(NT):
                sl = slice(t * 128, (t + 1) * 128)
                nc.tensor.matmul(m_ps[:, 2 * t:2 * t + 2],
                                 xt_sb[:, 0, sl], u1_sb[:, k, 0, :],
                                 start=True, stop=False)
                nc.tensor.matmul(m_ps[:, 2 * t:2 * t + 2],
                                 xt_sb[:, 1, sl], u1_sb[:, k, 1, :],
                                 start=False, stop=False)
                nc.tensor.matmul(m_ps[:, 2 * t:2 * t + 2],
                                 agT[:, 0, sl], u2_sb[:, k, 0, :],
                                 start=False, stop=False)
                nc.tensor.matmul(m_ps[:, 2 * t:2 * t + 2],
                                 agT[:, 1, sl], u2_sb[:, k, 1, :],
                                 start=False, stop=True)
            beta_sb = pwork.tile([128, NT], F32, tag="beta")
            nc.scalar.activation(beta_sb[:], m_ps[:, 0:2 * NT:2], AF.Sigmoid,
                                 bias=kb_sb[:, k:k + 1])

            # ---- per row-tile: fused projections + gate + LN stats ----
            mv = pwork.tile([128, 2 * NT], F32, tag="mv")
            h_all = pout.tile([128, NT, C], F32, tag="hall")
            for t in range(NT):
                sl = slice(t * 128, (t + 1) * 128)
                da = ps_da.tile([128, 2, C], F32, tag="da")
                pd = da[:, 0, :]
                pa = da[:, 1, :]
                nc.tensor.matmul(pd[:], xt_sb[:, 0, sl], wf_sb[:, k, 0, :],
                                 start=True, stop=False)
                nc.tensor.matmul(pd[:], xt_sb[:, 1, sl], wf_sb[:, k, 1, :],
                                 start=False, stop=False)
                nc.tensor.matmul(pd[:], agT[:, 0, sl], wd_sb[:, k, 0, :],
                                 start=False, stop=False)
                nc.tensor.matmul(pd[:], agT[:, 1, sl], wd_sb[:, k, 1, :],
                                 start=False, stop=False)
                nc.tensor.matmul(pd[:], ones_sb[:], bz_sb[:, k, 0, :],
                                 start=False, stop=True)
                nc.tensor.matmul(pa[:], agT[:, 0, sl], wa_sb[:, k, 0, :],
                                 start=True, stop=False)
                nc.tensor.matmul(pa[:], agT[:, 1, sl], wa_sb[:, k, 1, :],
                                 start=False, stop=False)
                nc.tensor.matmul(pa[:], ones_sb[:], bz_sb[:, k, 1, :],
                                 start=False, stop=True)
                pa_sb = pwork.tile([128, C], F32, tag="pasb")
                nc.scalar.copy(pa_sb[:], pa[:])
                nc.vector.scalar_tensor_tensor(h_all[:, t, :], pd[:],
                                               beta_sb[:, t:t + 1], pa_sb[:],
                                               ALU.mult, ALU.add)
                stats = pwork.tile([128, 6], F32, tag="stats")
                nc.vector.bn_stats(stats[:], h_all[:, t, :])
                nc.vector.bn_aggr(mv[:, 2 * t:2 * t + 2], stats[:])

            # LN tail in two halves so the first half's relus + output DMA
            # drain while the second half's bn stats are still in flight.
            out_sb = pout.tile([128, NT, C], BF16, tag="out")
            HH = NT // 2
            for hlf in range(2):
                t0 = hlf * HH
                sd = pwork.tile([128, HH], F32, tag=f"sd{hlf}")
                nc.scalar.activation(sd[:], mv[:, 2 * t0 + 1:2 * (t0 + HH):2],
                                     AF.Sqrt, bias=eps_t[:, 0:1])
                rs2 = pwork.tile([128, HH], F32, tag=f"rs2{hlf}")
                nc.vector.reciprocal(rs2[:], sd[:])
                ms = pwork.tile([128, HH], F32, tag=f"ms{hlf}")
                nc.vector.scalar_tensor_tensor(
                    ms[:], mv[:, 2 * t0:2 * (t0 + HH):2], -1.0, rs2[:],
                    ALU.mult, ALU.mult)
                for i in range(HH):
                    t = t0 + i
                    if ln_trivial:
                        nc.scalar.activation(out_sb[:, t, :], h_all[:, t, :],
                                             AF.Relu, bias=ms[:, i:i + 1],
                                             scale=rs2[:, i:i + 1])
                    else:
                        z_t = pwork.tile([128, C], F32, tag="z")
                        nc.scalar.activation(z_t[:], h_all[:, t, :], AF.Copy,
                                             bias=0.0, scale=rs2[:, i:i + 1])
                        zb = pwork.tile([128, C], F32, tag="zb")
                        nc.vector.tensor_scalar(zb[:], z_t[:], ms[:, i:i + 1],
                                                None, ALU.add)
                        zg = pwork.tile([128, C], F32, tag="zg")
                        nc.vector.tensor_tensor(zg[:], zb[:], gm_sb[:, k, :],
                                                ALU.mult)
                        za = pwork.tile([128, C], F32, tag="za")
                        nc.vector.tensor_tensor(za[:], zg[:], bt_sb[:, k, :],
                                                ALU.add)
                        nc.vector.tensor_scalar_max(out_sb[:, t, :], za[:],
                                                    0.0)
                # out DMA on the ACT queue: emitted right after this half's
                # relus (same engine) so it never stalls the SP input queue
                nc.scalar.dma_start(out[k][:, t0:t0 + HH, :],
                                    out_sb[:, t0:t0 + HH, :])

    nc.compile()
    return nc


def _prep_inputs(distribution_edge, feature_node, modal_id, W_feat, b_feat,
                 W_raw, b_raw, W_beta, b_beta, ln_gamma, ln_beta):
    de = np.ascontiguousarray(distribution_edge, dtype=np.float32)
    x = np.ascontiguousarray(feature_node, dtype=np.float32)
    Wf = np.asarray(W_feat, np.float32)
    bf = np.asarray(b_feat, np.float32)
    Wr = np.asarray(W_raw, np.float32)
    br = np.asarray(b_raw, np.float32)
    Wb = np.asarray(W_beta, np.float32)
    bb = np.asarray(b_beta, np.float32)
    g = np.asarray(ln_gamma, np.float32)
    be = np.asarray(ln_beta, np.float32)

    ln_trivial = bool(np.all(g == 1.0) and np.all(be == 0.0))

    # folded gate params (1/S folded into u2)
    u1 = np.stack([Wf[i] @ (Wb[i][:C] + Wb[i][2 * C:]) for i in range(M)])
    u2 = np.stack([Wr[i] @ (Wb[i][C:2 * C] - Wb[i][2 * C:]) / S
                   for i in range(M)])
    kk = np.array([bb[i] + bf[i] @ (Wb[i][:C] + Wb[i][2 * C:])
                   + br[i] @ (Wb[i][C:2 * C] - Wb[i][2 * C:])
                   for i in range(M)], np.float32)

    halves = n // RPC  # 2 chunks per block
    rr = np.arange(RPC)
    in_maps = []
    for c in range(NCORES):
        eT_c = np.empty((CPC, 128, NK, RPC), NP_F8)
        xb_c = np.empty((CPC, 128, NK, C), NP_F8)
        xt_c = np.empty((CPC, 128, 2, RPC), NP_BF16)
        wf_c = np.empty((128, CPC, 2, C), NP_BF16)
        wd_c = np.empty((128, CPC, 2, C), NP_BF16)
        wa_c = np.empty((128, CPC, 2, C), NP_BF16)
        u1_c = np.zeros((128, CPC, 2, 2), NP_BF16)
        u2_c = np.zeros((128, CPC, 2, 2), NP_BF16)
        kb_c = np.empty((128, CPC), np.float32)
        bz_c = np.empty((1, CPC, 2, C), NP_BF16)
        gm_c = np.empty((128, CPC, C), np.float32)
        bt_c = np.empty((128, CPC, C), np.float32)
        for k in range(CPC):
            g_idx = c * CPC + k               # global chunk id
            b_idx = g_idx // (M * halves)
            i_idx = (g_idx // halves) % M
            half = g_idx % halves
            r0 = i_idx * n + half * RPC       # first global row in batch b
            blk = de[b_idx, r0:r0 + RPC,
                     i_idx * n:(i_idx + 1) * n].copy()  # [RPC, n]
            blk[rr, half * RPC + rr] = 0.0    # zero self-edges
            rs = np.maximum(np.abs(blk).sum(axis=1), EPS_L1)
            eTs = (blk * (S / rs)[:, None]).T           # [n(j), RPC(rows)]
            eT_c[k] = eTs.astype(NP_F8).reshape(NK, 128, RPC).transpose(1, 0, 2)
            xblk = x[b_idx, i_idx * n:(i_idx + 1) * n, :]
            xb_c[k] = xblk.astype(NP_F8).reshape(NK, 128, C).transpose(1, 0, 2)
            xt_c[k] = (x[b_idx, r0:r0 + RPC, :].T.astype(NP_BF16)
                       .reshape(2, 128, RPC).transpose(1, 0, 2))
            wf_c[:, k] = Wf[i_idx].astype(NP_BF16).reshape(2, 128, C).transpose(1, 0, 2)
            wd_c[:, k] = (-Wr[i_idx] / S).astype(NP_BF16).reshape(2, 128, C).transpose(1, 0, 2)
            wa_c[:, k] = (Wr[i_idx] / S).astype(NP_BF16).reshape(2, 128, C).transpose(1, 0, 2)
            u1_c[:, k, :, 0] = u1[i_idx].astype(NP_BF16).reshape(2, 128).T
            u2_c[:, k, :, 0] = u2[i_idx].astype(NP_BF16).reshape(2, 128).T
            kb_c[:, k] = kk[i_idx]
            bz_c[0, k, 0] = (bf[i_idx] - br[i_idx]).astype(NP_BF16)
            bz_c[0, k, 1] = br[i_idx].astype(NP_BF16)
            gm_c[:, k] = g[i_idx][None, :]
            bt_c[:, k] = be[i_idx][None, :]
        im = dict(eTd=eT_c, xbd=xb_c, xtd=xt_c, wfd=wf_c, wdd=wd_c,
                  wad=wa_c, u1d=u1_c, u2d=u2_c, kbd=kb_c, bzd=bz_c,
                  onesr=np.ones((1, 128), NP_BF16))
        if not ln_trivial:
            im["gmd"] = gm_c
            im["btd"] = bt_c
        in_maps.append(im)
    return in_maps, ln_trivial


def kernel(**inputs) -> np.ndarray:
    in_maps, ln_trivial = _prep_inputs(**inputs)
    if ln_trivial not in _cache:
        _cache[ln_trivial] = _build(ln_trivial)
    nc = _cache[ln_trivial]
    res = run_bass_kernel_spmd(nc, in_maps, core_ids=list(range(NCORES)))
    out = np.empty((B * N, C), np.float32)
    for c in range(NCORES):
        o = np.asarray(res.results[c]["out"])  # [CPC, 128, NT, C] bf16
        o = o.astype(np.float32).transpose(0, 2, 1, 3).reshape(CPC * RPC, C)
        out[c * CPC * RPC:(c + 1) * CPC * RPC] = o
    return out.reshape(B, N, C)


# revision 17
# speedup vs baseline: 1.1643x; 1.0582x over previous
"""Trainium2 Bass kernel for nn_D2FAgg (block-diagonal GNN message passing).

Sharding: B*N = 24576 output rows -> 24 chunks of 1024 rows; 3 chunks/core
across 8 cores. Each chunk belongs to one (batch, modality) block of 2048
nodes.

Host prep folds the masked L1 row-normalization into the edge block
(eTs = (e*diag_mask/rowsum).T * S, fp8 e4m3) and pre-projects the node
features through W_raw (xw = x@W_r, fp8) plus the gate vector (x@u2 as an
extra column).  The device then computes everything in row-orientation --
rows of the chunk are PSUM partitions -- with no transposes at all:

  pa[row, 0:256] = S*(aggr+b_r)  = sum_j eTs[j,row]*xw[j,:] + S*b_r  (PE fp8
                   DoubleRow, K=2048, + u1/bias matmuls in the same group)
  pa[row, 256]   = S*(m1+m2)      (gate logit, same accumulation group)
  pd[row, 0:256] = S*feat         = xt.T@(S*W_f) + S*b_f             (PE bf16)
  beta/omb       = sigmoid(+-pa[:,256]/S +- K)                       (ACT)
  u              = beta * pd                                         (ACT copy)
  h' = S*h       = pa*omb + u;  LayerNorm is scale-invariant, so
  out            = relu((h'-mean)*rsqrt(var+eps))                    (DVE+ACT)
"""
import numpy as np
import ml_dtypes
from contextlib import ExitStack

import concourse.bacc as bacc
import concourse.mybir as mybir
import concourse.tile as tile
from concourse.bass_utils import run_bass_kernel_spmd

F32 = mybir.dt.float32
BF16 = mybir.dt.bfloat16
F8 = mybir.dt.float8e4
AF = mybir.ActivationFunctionType
ALU = mybir.AluOpType
DR = mybir.MatmulPerfMode.DoubleRow

NP_F8 = ml_dtypes.float8_e4m3
NP_BF16 = ml_dtypes.bfloat16

B, N, C = 4, 6144, 256
M = 3
n = N // M                      # 2048 nodes per modality block
NCORES = 8
RPC = 1024                      # rows per chunk
CPC = (B * N) // (NCORES * RPC)  # chunks per core = 3
NK = n // 128                   # 16 j-tiles per chunk
NT = RPC // 128                 # 8 row-tiles per chunk
NPC = 4                         # eT DMA pieces per chunk (4 k-tiles each)
CW = 264                        # padded xw width (256 aggr + 1 gate + pad)
GW = 258                        # pa accumulation width (256 aggr + logit + pad)
EPS_L1, EPS_LN = 1e-12, 1e-5
S = 2048.0                      # fp8 pre-scale for normalized edges

_cache = {}


def _build(ln_trivial: bool):
    nc = bacc.Bacc("TRN2", target_bir_lowering=False, debug=False,
                   num_devices=NCORES)
    eTd = nc.declare_dram_parameter("eTd", [CPC, 128, NK, RPC], F8,
                                    isOutput=False)
    xwd = nc.declare_dram_parameter("xwd", [CPC, 128, NK, CW], F8,
                                    isOutput=False)
    xtd = nc.declare_dram_parameter("xtd", [CPC, 128, 2, RPC], BF16,
                                    isOutput=False)
    wfd = nc.declare_dram_parameter("wfd", [128, CPC, 2, C], BF16,
                                    isOutput=False)
    u1d = nc.declare_dram_parameter("u1d", [128, CPC, 2, CW], BF16,
                                    isOutput=False)
    kbd = nc.declare_dram_parameter("kbd", [128, CPC, 2], F32, isOutput=False)
    bzd = nc.declare_dram_parameter("bzd", [1, CPC, 2, CW], BF16,
                                    isOutput=False)
    onesr = nc.declare_dram_parameter("onesr", [1, 128], BF16, isOutput=False)
    if not ln_trivial:
        gmd = nc.declare_dram_parameter("gmd", [128, CPC, C], F32,
                                        isOutput=False)
        btd = nc.declare_dram_parameter("btd", [128, CPC, C], F32,
                                        isOutput=False)
    out = nc.declare_dram_parameter("out", [CPC, 128, NT, C], BF16,
                                    isOutput=True)

    with ExitStack() as ctx:
        tc = ctx.enter_context(tile.TileContext(nc))
        const = ctx.enter_context(tc.tile_pool(name="const", bufs=1))
        px = ctx.enter_context(tc.tile_pool(name="px", bufs=2))
        pe_pool = ctx.enter_context(tc.tile_pool(name="pe", bufs=8))
        pwork = ctx.enter_context(tc.tile_pool(name="pwork", bufs=4))
        pout = ctx.enter_context(tc.tile_pool(name="pout", bufs=2))
        ps_da = ctx.enter_context(tc.tile_pool(name="psda", bufs=4,
                                               space="PSUM"))

        # once-loaded constants / weights (ACT HWDGE queue, off the SP queue)
        ones_sb = const.tile([1, 128], BF16)
        nc.scalar.dma_start(ones_sb[:], onesr[:])
        eps_t = const.tile([128, 1], F32)
        nc.vector.memset(eps_t[:], EPS_LN)
        wf_sb = const.tile([128, CPC, 2, C], BF16)
        nc.scalar.dma_start(wf_sb[:], wfd[:])
        u1_sb = const.tile([128, CPC, 2, CW], BF16)
        nc.scalar.dma_start(u1_sb[:], u1d[:])
        kb_sb = const.tile([128, CPC, 2], F32)
        nc.scalar.dma_start(kb_sb[:], kbd[:])
        bz_sb = const.tile([1, CPC, 2, CW], BF16)
        nc.scalar.dma_start(bz_sb[:], bzd[:])
        if not ln_trivial:
            gm_sb = const.tile([128, CPC, C], F32)
            nc.scalar.dma_start(gm_sb[:], gmd[:])
            bt_sb = const.tile([128, CPC, C], F32)
            nc.scalar.dma_start(bt_sb[:], btd[:])

        for k in range(CPC):
            xw_sb = px.tile([128, NK, CW], F8, tag="xw")
            nc.sync.dma_start(xw_sb[:], xwd[k])
            ets = []
            for pc in range(NPC):
                et = pe_pool.tile([128, 4, RPC], F8, tag="et")
                nc.sync.dma_start(et[:], eTd[k][:, 4 * pc:4 * pc + 4, :])
                ets.append(et)
            xt_sb = px.tile([128, 2, RPC], BF16, tag="xt")
            nc.sync.dma_start(xt_sb[:], xtd[k])

            mv = pwork.tile([128, 2 * NT], F32, tag="mv")
            h_all = pout.tile([128, NT, C], F32, tag="hall")
            out_sb = pout.tile([128, NT, C], BF16, tag="out")
            for t in range(NT):
                sl = slice(t * 128, (t + 1) * 128)
                da = ps_da.tile([128, 2, 512], F32, tag="da")
                pa = da[:, 0, 0:GW]
                pd = da[:, 1, 0:C]
                # pa group: fp8 DoubleRow aggregation (incl. gate logit col)
                # + u1 matvec + S*b_r bias, all in one accumulation group
                for pc in range(NPC):
                    for jj in range(2):
                        kt = 4 * pc + 2 * jj
                        nc.tensor.matmul(
                            pa[:],
                            ets[pc][:, 2 * jj:2 * jj + 2, sl],
                            xw_sb[:, kt:kt + 2, 0:GW],
                            start=(pc == 0 and jj == 0), stop=False,
                            perf_mode=DR)
                nc.tensor.matmul(pa[:], xt_sb[:, 0, sl],
                                 u1_sb[:, k, 0, 0:GW],
                                 start=False, stop=False)
                nc.tensor.matmul(pa[:], xt_sb[:, 1, sl],
                                 u1_sb[:, k, 1, 0:GW],
                                 start=False, stop=False)
                nc.tensor.matmul(pa[:], ones_sb[:], bz_sb[:, k, 0, 0:GW],
                                 start=False, stop=True)
                # pd group: S*feat
                nc.tensor.matmul(pd[:], xt_sb[:, 0, sl], wf_sb[:, k, 0, :],
                                 start=True, stop=False)
                nc.tensor.matmul(pd[:], xt_sb[:, 1, sl], wf_sb[:, k, 1, :],
                                 start=False, stop=False)
                nc.tensor.matmul(pd[:], ones_sb[:], bz_sb[:, k, 1, 0:C],
                                 start=False, stop=True)
                # gate scalars from the logit column
                beta_t = pwork.tile([128, 1], F32, tag="beta")
                nc.scalar.activation(beta_t[:], da[:, 0, 256:257], AF.Sigmoid,
                                     bias=kb_sb[:, k, 0:1], scale=1.0 / S)
                omb_t = pwork.tile([128, 1], F32, tag="omb")
                nc.scalar.activation(omb_t[:], da[:, 0, 256:257], AF.Sigmoid,
                                     bias=kb_sb[:, k, 1:2], scale=-1.0 / S)
                # u = beta * (S*feat) ; h' = omb * (S*aggr_full) + u
                u_t = pwork.tile([128, C], F32, tag="u")
                nc.scalar.activation(u_t[:], pd[:], AF.Copy, bias=0.0,
                                     scale=beta_t[:, 0:1])
                nc.vector.scalar_tensor_tensor(h_all[:, t, :], pa[:, 0:C],
                                               omb_t[:, 0:1], u_t[:],
                                               ALU.mult, ALU.add)
                stats = pwork.tile([128, 6], F32, tag="stats")
                nc.vector.bn_stats(stats[:], h_all[:, t, :])
                nc.vector.bn_aggr(mv[:, 2 * t:2 * t + 2], stats[:])

                # LN tail per half so outputs drain early
                if t % (NT // 2) == NT // 2 - 1:
                    hlf = t // (NT // 2)
                    HH = NT // 2
                    t0 = hlf * HH
                    sd = pwork.tile([128, HH], F32, tag=f"sd{hlf}")
                    nc.scalar.activation(sd[:],
                                         mv[:, 2 * t0 + 1:2 * (t0 + HH):2],
                                         AF.Sqrt, bias=eps_t[:, 0:1])
                    rs2 = pwork.tile([128, HH], F32, tag=f"rs2{hlf}")
                    nc.vector.reciprocal(rs2[:], sd[:])
                    ms = pwork.tile([128, HH], F32, tag=f"ms{hlf}")
                    nc.vector.scalar_tensor_tensor(
                        ms[:], mv[:, 2 * t0:2 * (t0 + HH):2], -1.0, rs2[:],
                        ALU.mult, ALU.mult)
                    for i in range(HH):
                        tt = t0 + i
                        if ln_trivial:
                            nc.scalar.activation(out_sb[:, tt, :],
                                                 h_all[:, tt, :], AF.Relu,
                                                 bias=ms[:, i:i + 1],
                                                 scale=rs2[:, i:i + 1])
                        else:
                            z_t = pwork.tile([128, C], F32, tag="z")
                            nc.scalar.activation(z_t[:], h_all[:, tt, :],
                                                 AF.Copy, bias=0.0,
                                                 scale=rs2[:, i:i + 1])
                            zb = pwork.tile([128, C], F32, tag="zb")
                            nc.vector.tensor_scalar(zb[:], z_t[:],
                                                    ms[:, i:i + 1], None,
                                                    ALU.add)
                            zg = pwork.tile([128, C], F32, tag="zg")
                            nc.vector.tensor_tensor(zg[:], zb[:],
                                                    gm_sb[:, k, :], ALU.mult)
                            za = pwork.tile([128, C], F32, tag="za")
                            nc.vector.tensor_tensor(za[:], zg[:],
                                                    bt_sb[:, k, :], ALU.add)
                            nc.vector.tensor_scalar_max(out_sb[:, tt, :],
                                                        za[:], 0.0)
                    # out DMA on the ACT queue (never stalls SP input queue)
                    nc.scalar.dma_start(out[k][:, t0:t0 + HH, :],
                                        out_sb[:, t0:t0 + HH, :])

    nc.compile()
    return nc


def _prep_inputs(distribution_edge, feature_node, modal_id, W_feat, b_feat,
                 W_raw, b_raw, W_beta, b_beta, ln_gamma, ln_beta):
    de = np.ascontiguousarray(distribution_edge, dtype=np.float32)
    x = np.ascontiguousarray(feature_node, dtype=np.float32)
    Wf = np.asarray(W_feat, np.float32)
    bf = np.asarray(b_feat, np.float32)
    Wr = np.asarray(W_raw, np.float32)
    br = np.asarray(b_raw, np.float32)
    Wb = np.asarray(W_beta, np.float32)
    bb = np.asarray(b_beta, np.float32)
    g = np.asarray(ln_gamma, np.float32)
    be = np.asarray(ln_beta, np.float32)

    ln_trivial = bool(np.all(g == 1.0) and np.all(be == 0.0))

    # folded gate params
    u1 = np.stack([Wf[i] @ (Wb[i][:C] + Wb[i][2 * C:]) for i in range(M)])
    u2 = np.stack([Wr[i] @ (Wb[i][C:2 * C] - Wb[i][2 * C:]) for i in range(M)])
    kk = np.array([bb[i] + bf[i] @ (Wb[i][:C] + Wb[i][2 * C:])
                   + br[i] @ (Wb[i][C:2 * C] - Wb[i][2 * C:])
                   for i in range(M)], np.float32)

    halves = n // RPC  # 2 chunks per block
    rr = np.arange(RPC)
    in_maps = []
    for c in range(NCORES):
        eT_c = np.empty((CPC, 128, NK, RPC), NP_F8)
        xw_c = np.zeros((CPC, 128, NK, CW), NP_F8)
        xt_c = np.empty((CPC, 128, 2, RPC), NP_BF16)
        wf_c = np.empty((128, CPC, 2, C), NP_BF16)
        u1_c = np.zeros((128, CPC, 2, CW), NP_BF16)
        kb_c = np.empty((128, CPC, 2), np.float32)
        bz_c = np.zeros((1, CPC, 2, CW), NP_BF16)
        gm_c = np.empty((128, CPC, C), np.float32)
        bt_c = np.empty((128, CPC, C), np.float32)
        for k in range(CPC):
            g_idx = c * CPC + k               # global chunk id
            b_idx = g_idx // (M * halves)
            i_idx = (g_idx // halves) % M
            half = g_idx % halves
            r0 = i_idx * n + half * RPC       # first global row in batch b
            blk = de[b_idx, r0:r0 + RPC,
                     i_idx * n:(i_idx + 1) * n].copy()  # [RPC, n]
            blk[rr, half * RPC + rr] = 0.0    # zero self-edges
            rs = np.maximum(np.abs(blk).sum(axis=1), EPS_L1)
            eTs = (blk * (S / rs)[:, None]).T           # [n(j), RPC(rows)]
            eT_c[k] = eTs.astype(NP_F8).reshape(NK, 128, RPC).transpose(1, 0, 2)
            xblk = x[b_idx, i_idx * n:(i_idx + 1) * n, :]   # [n, C]
            xw = np.empty((n, CW), np.float32)
            xw[:, 0:C] = xblk @ Wr[i_idx]
            xw[:, C] = xblk @ u2[i_idx]
            xw[:, C + 1:] = 0.0
            xw_c[k] = xw.astype(NP_F8).reshape(NK, 128, CW).transpose(1, 0, 2)
            xt_c[k] = (x[b_idx, r0:r0 + RPC, :].T.astype(NP_BF16)
                       .reshape(2, 128, RPC).transpose(1, 0, 2))
            wf_c[:, k] = (S * Wf[i_idx]).astype(NP_BF16).reshape(
                2, 128, C).transpose(1, 0, 2)
            u1_c[:, k, :, C] = (S * u1[i_idx]).astype(NP_BF16).reshape(2, 128).T
            kb_c[:, k, 0] = kk[i_idx]
            kb_c[:, k, 1] = -kk[i_idx]
            bz_c[0, k, 0, 0:C] = (S * br[i_idx]).astype(NP_BF16)
            bz_c[0, k, 1, 0:C] = (S * bf[i_idx]).astype(NP_BF16)
            gm_c[:, k] = g[i_idx][None, :]
            bt_c[:, k] = be[i_idx][None, :]
        im = dict(eTd=eT_c, xwd=xw_c, xtd=xt_c, wfd=wf_c, u1d=u1_c,
                  kbd=kb_c, bzd=bz_c, onesr=np.ones((1, 128), NP_BF16))
        if not ln_trivial:
            im["gmd"] = gm_c
            im["btd"] = bt_c
        in_maps.append(im)
    return in_maps, ln_trivial


def kernel(**inputs) -> np.ndarray:
    in_maps, ln_trivial = _prep_inputs(**inputs)
    if ln_trivial not in _cache:
        _cache[ln_trivial] = _build(ln_trivial)
    nc = _cache[ln_trivial]
    res = run_bass_kernel_spmd(nc, in_maps, core_ids=list(range(NCORES)))
    out = np.empty((B * N, C), np.float32)
    for c in range(NCORES):
        o = np.asarray(res.results[c]["out"])  # [CPC, 128, NT, C] bf16
        o = o.astype(np.float32).transpose(0, 2, 1, 3).reshape(CPC * RPC, C)
        out[c * CPC * RPC:(c + 1) * CPC * RPC] = o
    return out.reshape(B, N, C)
